# revision 6
# baseline (speedup 1.0000x reference)
"""Trainium2 Bass kernel v2: batched attention (B=8, S=4096, D=64), fp32.

out[b] = softmax(q[b] @ k[b].T / sqrt(D), axis=keys) @ v[b] * mask[b, :, None]

Data-parallel over batch: one batch element per NeuronCore, 8 cores.

Per-core algorithm:
  1. Q, K transposed to d-major fp16, PRESCALED by sqrt(2^10*log2e/8) each so
     the QK^T psum value is s = 2^10 * log2(e) * (q.k)/8 -- the exact fixed-
     point argument both exp paths want.
  2. scoresT[k, q] slabs ([128 keys x 512 q] psum) via one-shot half-array
     matmul pairs (even/odd k-tiles on the two 64-row PE halves).
  3. exp split across TWO engines per 3-slab chunk, alternating:
     - ScalarE ('A'): PT = Exp(ln2/1024 * s) -> fp16
     - VectorE ('V'): custom 2-op fast-exp: I16 = 2^10*(15+round(t)+f+h(f)),
       h fitted over {1, f, f^2, |f|}, int16 store == fp16 bits of e^x
       (bit-exact vs numpy emulation on HW; max P rel err 2.9e-3)
     - VectorE ('W'): 1-op quad-only variant (max P rel err ~1.6%, cheap).
     A/W/V mix balances ACT vs DVE throughput against the 2e-2 error gate;
     ACT and DVE write separate PT tiles (per-tile dep tracking).
  4. PV swapped: PT subtile [128k x 128q] STATIONARY, moving V' [128k x 65]
     (V padded with a ones column -> denominator lands in psum col 64).
     Accumulate over 32 k-tiles into psum [128q, 4, 65] per 512-q chunk;
     each sub-chain contiguous (interleaving start/stop groups within one
     psum bank corrupts accumulation; across banks it is fine but slower
     here). PV pieces are emitted BETWEEN exp chunks of the next q-chunk so
     the PE works while ACT/DVE chew on exp.
  5. Epilogue (no PE): rec = 1/denom (DVE), rm = rec*mask (DVE),
     osb = psum * rm (GpSimd), DMA out. Output is already [q, d] natural.
"""

import sys

if "/opt/trn_rl_repo" not in sys.path:
    sys.path.insert(0, "/opt/trn_rl_repo")

import math
from contextlib import ExitStack

import numpy as np

import concourse.bass as bass
import concourse.mybir as mybir
import concourse.tile as tile
from concourse import bacc
from concourse.masks import make_identity

F32 = mybir.dt.float32
FP16 = mybir.dt.float16
I16 = mybir.dt.int16

B = 8          # batch == number of cores
S = 4096       # sequence length
D = 64         # head dim
P = 128        # partitions
NKT = S // P   # 32 k-tiles of 128 keys
QCHUNK = 512   # query chunk (psum-bank-sized scores slabs)
NQC = S // QCHUNK          # 8 query chunks
NSUB = QCHUNK // P         # 4 q-subtiles per chunk

# ---- fast-exp fixed-point scaling ------------------------------------------
# psum scores s = ALPHA * (q.k); t = s/2^10 = log2(e) * (q.k)/8
ALPHA = 1024.0 * math.log2(math.e) / 8.0
CSCALE = math.sqrt(ALPHA)            # folded into each of Q and K
ACT_SCALE = math.log(2.0) / 1024.0   # ScalarE: exp(ACT_SCALE * s) = e^{qk/8}
CBIG = float(3 * 2**32)              # fp32 round-to-nearest at 2^10 granularity
# minimax fit of h(f) = {f>=0: 2^f-1-f ; f<0: 2^(f+1)-2-f} on [-1/2,1/2]
# over {1, f, f^2, |f|}: max resid 2.75e-3
C_H0 = -2.37450629e-04
C_H1 = 5.50407949e-03
C_H2 = 3.44100112e-01
C_HA = -3.43148039e-01
K_C1 = float(C_H2 / 1024.0)            # op1 s1
K_C2 = float(1.0 + C_H1)               # op1 imm2
K_C0K = float((C_H0 + 15.0) * 1024.0)  # op1 C3->Latch(Src1) [P,1] value
K_CA = float(C_HA)                     # op2 s1 literal

# per-q-chunk exp engine pattern: (engine, nslabs) summing NKT=32.
# 'A' = ScalarE activation, 'V' = VectorE custom 2-op fast-exp
EXP_PATTERN = ([("A", 2), ("W", 2)] * 6 + [("A", 2), ("V", 2)] * 2)
PIPELINE = True
SKIP_PV = False    # diagnostic: drop PV+epilogue (QK+exp pipeline only)
SKIP_QKEXP = False  # diagnostic: PV reads prologue-seeded PT; no QK/exp
# PV chain layout: "safe"  = sub-outer, contiguous chains, 1-bank psum tile
#                  "fast2" = 2 banks, pairs of chains interleaved kt-outer
#                  "fast4" = 4 banks, all 4 chains interleaved kt-outer
PV_MODE = "safe"
SC_BUFS = 3
PV_BUFS = 2  # default: 1 for fast4 else 2

# ---- custom DVE op registration --------------------------------------------
from concourse.dve_spec import (
    Spec, Src0, Src1, C0, C1, C2, C3, Zero, maxx,
    lower as _dve_lower, _has_src1, _spill_c3_to_src1,
)
from concourse.dve_ops import (
    DveOp, OPS as _DVE_OPS, CUSTOM_DVE_SPECS as _DVE_SPECS,
    _SUB_OPCODE_FOR_NAME as _DVE_ROWS,
)
from concourse.dve_uop import DveOpSpec


def _register_op(name, body):
    if name in _DVE_ROWS:
        return next(op for op in _DVE_OPS if op.name == name)
    spec = Spec(body=_spill_c3_to_src1(body))
    uops = _dve_lower(spec, ver="v3")
    row = 1 + len(_DVE_OPS)
    assert row < 0x20
    sha = DveOpSpec(name=name, opcode=row, uops=uops,
                    rd1_en=_has_src1(spec)).sha("v3")
    op = DveOp(name=name, spec=spec, subdim=False, uops_sha={"v3": sha})
    _DVE_OPS.append(op)
    _DVE_SPECS[name] = spec
    _DVE_ROWS[name] = row
    return op


def _exp16a_body():
    # m4 = ((F*C1 + C2)*F) + C3 ; F = s - ((s + C0) - C0)
    R = Src0 + C0
    G = R - C0
    F = Src0 - G
    return ((F * C1) + C2) * F + C3


def _exp16b_body():
    # I = (|F| * C1) + Src0 + G   (Src1 = scores stream, rank-3)
    R = Src1 + C0
    G = R - C0
    F = Src1 - G
    A = maxx(F, Zero - F)
    return (A * C1) + Src0 + G


def _exp16w_body():
    # single-op quadratic-only variant (no |f| kink term): cheap but coarser
    # (max P rel err ~1.6%); I = ((F*C1 + C2)*F + C3) + G
    R = Src0 + C0
    G = R - C0
    F = Src0 - G
    return (((F * C1) + C2) * F + C3) + G


EXP16A = _register_op("ANT_EXP16A", _exp16a_body())
EXP16B = _register_op("ANT_EXP16B", _exp16b_body())
EXP16W = _register_op("ANT_EXP16W", _exp16w_body())

# quad-only minimax coefficients (basis {1, f, f^2}), resid 2.25e-2
W_H0 = -0.02251146
W_H1 = 0.01399606
W_H2 = -0.31515363
KW_C1 = float(W_H2 / 1024.0)
KW_C2 = float(1.0 + W_H1)
KW_C0K = float((W_H0 + 15.0) * 1024.0)


# ---- kernel build ----------------------------------------------------------
def build_attention(ctx: ExitStack, tc: tile.TileContext,
                    q_ap, k_ap, v_ap, mask_ap, out_ap, reps=1):
    nc = tc.nc

    const_pool = ctx.enter_context(tc.tile_pool(name="const", bufs=1))
    io_pool = ctx.enter_context(tc.tile_pool(name="io", bufs=1))

    ident = const_pool.tile([P, P], F32, tag="ident", name="ident")
    make_identity(nc, ident)
    cst = const_pool.tile([P, 1], F32, tag="cst", name="cst")
    nc.gpsimd.memset(cst, K_C0K)
    cstw = const_pool.tile([P, 1], F32, tag="cstw", name="cstw")
    nc.gpsimd.memset(cstw, KW_C0K)

    # persistent SBUF tensors
    qt = [io_pool.tile([P, S // 2], FP16, tag=f"qt{h}", name=f"qt{h}")
          for h in range(2)]
    kt = [io_pool.tile([P, S // 2], FP16, tag=f"kt{h}", name=f"kt{h}")
          for h in range(2)]
    vp = io_pool.tile([P, NKT, D + 1], FP16, tag="vp", name="vp")
    msk = io_pool.tile([P, NKT], F32, tag="msk", name="msk")

    # ---- optional in-NEFF repetition of the ENTIRE body (timing) -----------
    loop_cm = None
    if reps > 1:
        loop_cm = tc.For_i(0, reps, 1, hint_engines=(
            mybir.EngineType.PE, mybir.EngineType.Activation,
            mybir.EngineType.DVE, mybir.EngineType.Pool))
        loop_cm.__enter__()

    # ---- prologue: load + transpose Q and K (scaled), build V' -------------
    stage_pool = ctx.enter_context(tc.tile_pool(name="stage", bufs=1))
    qn = stage_pool.tile([P, NKT, D], F32, tag="qn", name="qn")
    kn = stage_pool.tile([P, NKT, D], F32, tag="kn", name="kn")
    ones = stage_pool.tile([P, NKT], F32, tag="ones", name="ones")

    q_tiled = q_ap.rearrange("(t p) d -> p t d", p=P)
    k_tiled = k_ap.rearrange("(t p) d -> p t d", p=P)
    v_tiled = v_ap.rearrange("(t p) d -> p t d", p=P)
    mask_tiled = mask_ap.rearrange("1 (t p) -> p t", p=P)

    nc.sync.dma_start(qn[:], q_tiled)
    nc.sync.dma_start(kn[:], k_tiled)
    nc.sync.dma_start(msk, mask_tiled)
    nc.gpsimd.memset(ones, 1.0)
    nc.vector.tensor_copy(vp[:, :, D], ones)
    vn = stage_pool.tile([P, NKT, D], F32, tag="vn", name="vn")
    nc.sync.dma_start(vn[:], v_tiled)
    nc.vector.tensor_copy(vp[:, :, 0:D], vn)

    with tc.tile_pool(name="tpsum", bufs=4, space="PSUM") as tpsum_pool:
        # Transpose 4 input tiles [128, 64] into one psum bank [64, 512],
        # drain with the CSCALE prescale fused (alternate DVE/ACT).
        for half in range(2):
            for src_i, (src, dst) in enumerate(((qn, qt), (kn, kt))):
                for c in range(4):
                    ps = tpsum_pool.tile([D, 4 * P], F32, tag="tps", name="tps")
                    for j in range(4):
                        t = half * (NKT // 2) + c * 4 + j
                        nc.tensor.transpose(ps[:, j * P:(j + 1) * P],
                                            src[:, t, :], ident)
                    dcol = c * 4 * P
                    if (c + src_i) % 2 == 0:
                        nc.vector.tensor_scalar_mul(
                            dst[half][0:D, dcol:dcol + 4 * P], ps, CSCALE)
                    else:
                        nc.scalar.mul(
                            dst[half][0:D, dcol:dcol + 4 * P], ps, CSCALE)
            nc.sync.dma_start(qt[half][D:P, :], qt[half][0:D, :])
            nc.sync.dma_start(kt[half][D:P, :], kt[half][0:D, :])

    # ---- main loop ---------------------------------------------------------
    pv_bufs = PV_BUFS if PV_BUFS is not None else (1 if PV_MODE == "fast4" else 2)
    pt_pool = ctx.enter_context(tc.tile_pool(name="pt", bufs=2))
    sc_pool = ctx.enter_context(tc.tile_pool(name="sc", bufs=SC_BUFS,
                                             space="PSUM"))
    m4_pool = ctx.enter_context(tc.tile_pool(name="m4", bufs=2))
    pv_pool = ctx.enter_context(tc.tile_pool(
        name="pv", bufs=pv_bufs, space="PSUM"))
    osb_pool = ctx.enter_context(tc.tile_pool(name="osb", bufs=2))
    scal_pool = ctx.enter_context(tc.tile_pool(name="scal", bufs=4))

    # psum column offset of sub-chain accumulators & total pv tile columns
    if PV_MODE == "safe":
        PV_COLS = NSUB * (D + 1)
        pv_off = lambda sub: sub * (D + 1)
    elif PV_MODE == "fast2":
        PV_COLS = 2 * 512
        pv_off = lambda sub: (sub % 2) * 512 + (sub // 2) * (D + 1)
    else:  # fast4
        PV_COLS = 4 * 512
        pv_off = lambda sub: sub * 512

    out_tiled = out_ap.rearrange("(t p) d -> p t d", p=P)

    pt_tiles = {}   # qc -> ptt tile [P, NKT, QCHUNK] fp16
    pv_tiles = {}   # qc -> pv psum tile [P, NSUB, D+1]

    sink = None
    if SKIP_PV:
        sink = io_pool.tile([P, NQC, 32], FP16, tag="sink", name="sink")
    fixed_pt = None
    if SKIP_QKEXP:
        _fpt = io_pool.tile([P, NKT, QCHUNK], FP16, tag="fpt", name="fpt")
        for j in range(NKT // 2):
            nc.gpsimd.tensor_copy(
                _fpt.rearrange("p a b -> p (a b)")[:, j * 1024:(j + 1) * 1024],
                qn.rearrange("p a b -> p (a b)")[:, 0:1024])
        fixed_pt = {"A": _fpt, "D": _fpt}

    # chunk -> (engine-class tile key, slab offset within that tile);
    # kt -> same, for PV lhsT lookup. Engine classes: ACT ('A') vs DVE
    # ('V'/'W'), each writing its OWN pt tile so the two exp engines never
    # share a write target (tile-granular dep tracking would serialize them).
    chunk_starts = []
    kt_map = {}
    n_slab = {"A": 0, "D": 0}
    _kt = 0
    for _eng, _cl in EXP_PATTERN:
        key = "A" if _eng == "A" else "D"
        chunk_starts.append((key, n_slab[key]))
        for _j in range(_cl):
            kt_map[_kt + _j] = (key, n_slab[key] + _j)
        n_slab[key] += _cl
        _kt += _cl
    assert _kt == NKT

    def emit_exp_chunk(qc, c):
        q0 = qc * QCHUNK
        qt_half = qt[(2 * q0) // S]
        qcol = q0 % (S // 2)
        if c == 0:
            tiles = {}
            for key, cnt in n_slab.items():
                if cnt:
                    tiles[key] = pt_pool.tile([P, cnt, QCHUNK], FP16,
                                              tag=f"ptt{key}", name=f"ptt{key}")
            pt_tiles[qc] = tiles
        tiles = pt_tiles[qc]
        eng, clen = EXP_PATTERN[c]
        key, slab0 = chunk_starts[c]
        kt0 = sum(cl for _, cl in EXP_PATTERN[:c])
        scs = sc_pool.tile([P, clen * QCHUNK], F32, tag="sc", name="sc")
        for jj in range(clen):
            k_tile = kt0 + jj
            h = k_tile % 2
            kt_half = kt[(k_tile * P * 2) // S]
            kcol = (k_tile * P) % (S // 2)
            nc.tensor.matmul(
                scs[:, jj * QCHUNK:(jj + 1) * QCHUNK],
                lhsT=kt_half[h * D:(h + 1) * D, kcol:kcol + P],
                rhs=qt_half[h * D:(h + 1) * D, qcol:qcol + QCHUNK],
                start=True, stop=True,
            )
        dst = tiles[key][:, slab0:slab0 + clen, :]
        if eng == "A":
            nc.scalar.activation(
                dst.rearrange("p a b -> p (a b)"), scs,
                mybir.ActivationFunctionType.Exp, scale=ACT_SCALE)
        elif eng == "W":
            nc.vector._custom_dve(
                EXP16W, out=dst.rearrange("p a b -> p (a b)").bitcast(I16),
                in0=scs, in1=cstw,
                s0=CBIG, s1=KW_C1, imm2=KW_C2)
        else:
            m4 = m4_pool.tile([P, clen * QCHUNK], F32, tag="m4", name="m4")
            nc.vector._custom_dve(
                EXP16A, out=m4, in0=scs, in1=cst,
                s0=CBIG, s1=K_C1, imm2=K_C2)
            nc.vector._custom_dve(
                EXP16B,
                out=dst.bitcast(I16),
                in0=m4.rearrange("p (a b) -> p a b", a=clen),
                in1=scs.rearrange("p (a b) -> p a b", a=clen),
                s0=CBIG, s1=K_CA)

    def pv_mm(pv_ps, tiles, k_tile, sub):
        off = pv_off(sub)
        key, idx = kt_map[k_tile]
        nc.tensor.matmul(
            pv_ps[:, off:off + D + 1],
            lhsT=tiles[key][:, idx, sub * P:(sub + 1) * P],
            rhs=vp[:, k_tile, :],
            start=(k_tile == 0), stop=(k_tile == NKT - 1),
            skip_group_check=True,
        )

    def pv_pieces(qc):
        """List of ~16-MM emission closures in a chain-legal order."""
        tiles = fixed_pt if SKIP_QKEXP else pt_tiles.pop(qc)
        if SKIP_PV:
            def piece():
                for j, t in enumerate(tiles.values()):
                    nc.gpsimd.tensor_copy(sink[:, qc, 16 * j:16 * j + 16],
                                          t[:, 0, 0:16])
                if qc == NQC - 1:
                    nc.sync.dma_start(
                        out_tiled[:, 0:2, :],
                        sink.bitcast(F32).rearrange(
                            "p a b -> p (a b)").rearrange(
                            "p (t d) -> p t d", t=2))
            return [piece]
        pv_ps = pv_pool.tile([P, PV_COLS], F32, tag="pv", name="pv")
        pv_tiles[qc] = pv_ps
        pieces = []
        if PV_MODE == "safe":
            # contiguous chain per sub, split in kt halves
            for sub in range(NSUB):
                for half in range(2):
                    def piece(sub=sub, half=half):
                        for k_tile in range(half * 16, half * 16 + 16):
                            pv_mm(pv_ps, tiles, k_tile, sub)
                    pieces.append(piece)
        elif PV_MODE == "fast2":
            # two chains interleaved across two banks, two passes
            for pair in range(2):
                for quarter in range(4):
                    def piece(pair=pair, quarter=quarter):
                        for k_tile in range(quarter * 8, quarter * 8 + 8):
                            for sub in (2 * pair, 2 * pair + 1):
                                pv_mm(pv_ps, tiles, k_tile, sub)
                    pieces.append(piece)
        else:  # fast4: all four chains interleaved kt-outer
            for eighth in range(8):
                def piece(eighth=eighth):
                    for k_tile in range(eighth * 4, eighth * 4 + 4):
                        for sub in range(NSUB):
                            pv_mm(pv_ps, tiles, k_tile, sub)
                pieces.append(piece)
        return pieces

    def emit_epilogue(qc):
        pv_ps = pv_tiles.pop(qc)
        rs = scal_pool.tile([P, 2 * NSUB], F32, tag="rs", name="rs")
        for sub in range(NSUB):
            nc.vector.reciprocal(rs[:, sub:sub + 1],
                                 pv_ps[:, pv_off(sub) + D:pv_off(sub) + D + 1])
        nc.vector.tensor_mul(rs[:, NSUB:2 * NSUB], rs[:, 0:NSUB],
                             msk[:, qc * NSUB:(qc + 1) * NSUB])
        osb = osb_pool.tile([P, NSUB, D], F32, tag="osb", name="osb")
        for sub in range(NSUB):
            src = pv_ps[:, pv_off(sub):pv_off(sub) + D]
            if sub % 2 == 0:
                nc.scalar.activation(
                    osb[:, sub, :], src,
                    mybir.ActivationFunctionType.Copy,
                    scale=rs[:, NSUB + sub:NSUB + sub + 1],
                )
            else:
                nc.vector.tensor_scalar(
                    osb[:, sub, :], src,
                    rs[:, NSUB + sub:NSUB + sub + 1], None,
                    mybir.AluOpType.mult,
                )
        nc.sync.dma_start(
            out_tiled[:, qc * NSUB:(qc + 1) * NSUB, :], osb)

    n_chunks = len(EXP_PATTERN)
    if PIPELINE:
        # PV(s-1) pieces slot between QK/exp chunk emissions of step s so the
        # PE works through PV while ACT/DVE chew on exp(s).
        for s in range(NQC + 1):
            pieces = pv_pieces(s - 1) if s >= 1 else []
            if s < NQC and not SKIP_QKEXP:
                for c in range(n_chunks):
                    emit_exp_chunk(s, c)
                    pi = c - 2
                    if 0 <= pi < len(pieces):
                        pieces[pi]()
                for pi in range(max(0, n_chunks - 2), len(pieces)):
                    pieces[pi]()
            else:
                for piece in pieces:
                    piece()
            if s >= 1 and not SKIP_PV:
                emit_epilogue(s - 1)
    else:
        for s in range(NQC):
            if not SKIP_QKEXP:
                for c in range(n_chunks):
                    emit_exp_chunk(s, c)
            for piece in pv_pieces(s):
                piece()
            if not SKIP_PV:
                emit_epilogue(s)

    if loop_cm is not None:
        loop_cm.__exit__(None, None, None)


def build_program(reps=1):
    nc = bacc.Bacc("TRN2", target_bir_lowering=False, debug=False,
                   num_devices=B)
    q = nc.declare_dram_parameter("q", [S, D], F32, isOutput=False).ap()
    k = nc.declare_dram_parameter("k", [S, D], F32, isOutput=False).ap()
    v = nc.declare_dram_parameter("v", [S, D], F32, isOutput=False).ap()
    mask = nc.declare_dram_parameter("mask", [1, S], F32, isOutput=False).ap()
    out = nc.declare_dram_parameter("out", [S, D], F32, isOutput=True).ap()

    with tile.TileContext(nc) as tc, ExitStack() as ctx:
        build_attention(ctx, tc, q, k, v, mask, out, reps=reps)
    nc.compile()
    return nc


_NC_CACHE = None


def _get_nc():
    global _NC_CACHE
    if _NC_CACHE is None:
        _NC_CACHE = build_program()
    return _NC_CACHE


def make_in_maps(q, k, v, mask):
    return [
        {
            "q": np.ascontiguousarray(q[b], dtype=np.float32),
            "k": np.ascontiguousarray(k[b], dtype=np.float32),
            "v": np.ascontiguousarray(v[b], dtype=np.float32),
            "mask": np.ascontiguousarray(mask[b][None, :], dtype=np.float32),
        }
        for b in range(B)
    ]


def kernel(q, k, v, mask):
    from concourse.bass_utils import run_bass_kernel_spmd

    nc = _get_nc()
    res = run_bass_kernel_spmd(nc, make_in_maps(q, k, v, mask), list(range(B)))
    return np.stack([res.results[b]["out"] for b in range(B)])


if __name__ == "__main__":
    rng = np.random.default_rng(0)
    q = rng.standard_normal((B, S, D), dtype=np.float32)
    k = rng.standard_normal((B, S, D), dtype=np.float32)
    v = rng.standard_normal((B, S, D), dtype=np.float32)
    mask = np.ones((B, S), dtype=np.float32)
    out = kernel(q, k, v, mask)
    print("out", out.shape, out.dtype, float(np.abs(out).max()))


# revision 7
# speedup vs baseline: 1.2727x; 1.2727x over previous
"""Trainium2 Bass kernel v2: batched attention (B=8, S=4096, D=64), fp32.

out[b] = softmax(q[b] @ k[b].T / sqrt(D), axis=keys) @ v[b] * mask[b, :, None]

Data-parallel over batch: one batch element per NeuronCore, 8 cores.

Per-core algorithm:
  1. Q, K transposed to d-major fp16, PRESCALED by sqrt(2^10*log2e/8) each so
     the QK^T psum value is s = 2^10 * log2(e) * (q.k)/8 -- the exact fixed-
     point argument both exp paths want.
  2. scoresT[k, q] slabs ([128 keys x 512 q] psum) via one-shot half-array
     matmul pairs (even/odd k-tiles on the two 64-row PE halves).
  3. exp split across TWO engines per 3-slab chunk, alternating:
     - ScalarE ('A'): PT = Exp(ln2/1024 * s) -> fp16
     - VectorE ('V'): custom 2-op fast-exp: I16 = 2^10*(15+round(t)+f+h(f)),
       h fitted over {1, f, f^2, |f|}, int16 store == fp16 bits of e^x
       (bit-exact vs numpy emulation on HW; max P rel err 2.9e-3)
     - VectorE ('W'): 1-op quad-only variant (max P rel err ~1.6%, cheap).
     A/W/V mix balances ACT vs DVE throughput against the 2e-2 error gate;
     ACT and DVE write separate PT tiles (per-tile dep tracking).
  4. PV swapped: PT subtile [128k x 128q] STATIONARY, moving V' [128k x 65]
     (V padded with a ones column -> denominator lands in psum col 64).
     Accumulate over 32 k-tiles into psum [128q, 4, 65] per 512-q chunk;
     each sub-chain contiguous (interleaving start/stop groups within one
     psum bank corrupts accumulation; across banks it is fine but slower
     here). PV pieces are emitted BETWEEN exp chunks of the next q-chunk so
     the PE works while ACT/DVE chew on exp.
  5. Epilogue (no PE): rec = 1/denom (DVE), rm = rec*mask (DVE),
     osb = psum * rm (GpSimd), DMA out. Output is already [q, d] natural.
"""

import sys

if "/opt/trn_rl_repo" not in sys.path:
    sys.path.insert(0, "/opt/trn_rl_repo")

import math
from contextlib import ExitStack

import numpy as np

import concourse.bass as bass
import concourse.mybir as mybir
import concourse.tile as tile
from concourse import bacc
from concourse.masks import make_identity

F32 = mybir.dt.float32
FP16 = mybir.dt.float16
I16 = mybir.dt.int16

B = 8          # batch == number of cores
S = 4096       # sequence length
D = 64         # head dim
P = 128        # partitions
NKT = S // P   # 32 k-tiles of 128 keys
QCHUNK = 512   # query chunk (psum-bank-sized scores slabs)
NQC = S // QCHUNK          # 8 query chunks
NSUB = QCHUNK // P         # 4 q-subtiles per chunk

# ---- fast-exp fixed-point scaling ------------------------------------------
# psum scores s = ALPHA * (q.k); t = s/2^10 = log2(e) * (q.k)/8
ALPHA = 1024.0 * math.log2(math.e) / 8.0
CSCALE = math.sqrt(ALPHA)            # folded into each of Q and K
ACT_SCALE = math.log(2.0) / 1024.0   # ScalarE: exp(ACT_SCALE * s) = e^{qk/8}
CBIG = float(3 * 2**32)              # fp32 round-to-nearest at 2^10 granularity
# minimax fit of h(f) = {f>=0: 2^f-1-f ; f<0: 2^(f+1)-2-f} on [-1/2,1/2]
# over {1, f, f^2, |f|}: max resid 2.75e-3
C_H0 = -2.37450629e-04
C_H1 = 5.50407949e-03
C_H2 = 3.44100112e-01
C_HA = -3.43148039e-01
K_C1 = float(C_H2 / 1024.0)            # op1 s1
K_C2 = float(1.0 + C_H1)               # op1 imm2
K_C0K = float((C_H0 + 15.0) * 1024.0)  # op1 C3->Latch(Src1) [P,1] value
K_CA = float(C_HA)                     # op2 s1 literal

# per-q-chunk exp engine pattern: (engine, nslabs) summing NKT=32.
# 'A' = ScalarE activation, 'V' = VectorE custom 2-op fast-exp
EXP_PATTERN = ([("A", 2), ("W", 2)] * 6 + [("A", 2), ("V", 2)] * 2)
PIPELINE = True
SKIP_PV = False    # diagnostic: drop PV+epilogue (QK+exp pipeline only)
SKIP_QKEXP = False  # diagnostic: PV reads prologue-seeded PT; no QK/exp
# PV chain layout: "safe"  = sub-outer, contiguous chains, 1-bank psum tile
#                  "fast2" = 2 banks, pairs of chains interleaved kt-outer
#                  "fast4" = 4 banks, all 4 chains interleaved kt-outer
PV_MODE = "safe"
SC_BUFS = 3
PV_BUFS = 2  # default: 1 for fast4 else 2

# ---- custom DVE op registration --------------------------------------------
from concourse.dve_spec import (
    Spec, Src0, Src1, C0, C1, C2, C3, Zero, maxx,
    lower as _dve_lower, _has_src1, _spill_c3_to_src1,
)
from concourse.dve_ops import (
    DveOp, OPS as _DVE_OPS, CUSTOM_DVE_SPECS as _DVE_SPECS,
    _SUB_OPCODE_FOR_NAME as _DVE_ROWS,
)
from concourse.dve_uop import DveOpSpec


def _register_op(name, body):
    if name in _DVE_ROWS:
        return next(op for op in _DVE_OPS if op.name == name)
    spec = Spec(body=_spill_c3_to_src1(body))
    uops = _dve_lower(spec, ver="v3")
    row = 1 + len(_DVE_OPS)
    assert row < 0x20
    sha = DveOpSpec(name=name, opcode=row, uops=uops,
                    rd1_en=_has_src1(spec)).sha("v3")
    op = DveOp(name=name, spec=spec, subdim=False, uops_sha={"v3": sha})
    _DVE_OPS.append(op)
    _DVE_SPECS[name] = spec
    _DVE_ROWS[name] = row
    return op


def _exp16a_body():
    # m4 = ((F*C1 + C2)*F) + C3 ; F = s - ((s + C0) - C0)
    R = Src0 + C0
    G = R - C0
    F = Src0 - G
    return ((F * C1) + C2) * F + C3


def _exp16b_body():
    # I = (|F| * C1) + Src0 + G   (Src1 = scores stream, rank-3)
    R = Src1 + C0
    G = R - C0
    F = Src1 - G
    A = maxx(F, Zero - F)
    return (A * C1) + Src0 + G


def _exp16w_body():
    # single-op quadratic-only variant (no |f| kink term): cheap but coarser
    # (max P rel err ~1.6%); I = ((F*C1 + C2)*F + C3) + G
    R = Src0 + C0
    G = R - C0
    F = Src0 - G
    return (((F * C1) + C2) * F + C3) + G


EXP16A = _register_op("ANT_EXP16A", _exp16a_body())
EXP16B = _register_op("ANT_EXP16B", _exp16b_body())
EXP16W = _register_op("ANT_EXP16W", _exp16w_body())

# quad-only minimax coefficients (basis {1, f, f^2}), resid 2.25e-2
W_H0 = -0.02251146
W_H1 = 0.01399606
W_H2 = -0.31515363
KW_C1 = float(W_H2 / 1024.0)
KW_C2 = float(1.0 + W_H1)
KW_C0K = float((W_H0 + 15.0) * 1024.0)


# ---- kernel build ----------------------------------------------------------
def build_attention(ctx: ExitStack, tc: tile.TileContext,
                    q_ap, k_ap, v_ap, mask_ap, out_ap, reps=1):
    nc = tc.nc

    const_pool = ctx.enter_context(tc.tile_pool(name="const", bufs=1))
    io_pool = ctx.enter_context(tc.tile_pool(name="io", bufs=1))

    ident = const_pool.tile([P, P], F32, tag="ident", name="ident")
    make_identity(nc, ident)
    cst = const_pool.tile([P, 1], F32, tag="cst", name="cst")
    nc.gpsimd.memset(cst, K_C0K)
    cstw = const_pool.tile([P, 1], F32, tag="cstw", name="cstw")
    nc.gpsimd.memset(cstw, KW_C0K)

    # persistent SBUF tensors
    qt = [io_pool.tile([P, S // 2], FP16, tag=f"qt{h}", name=f"qt{h}")
          for h in range(2)]
    kt = [io_pool.tile([P, S // 2], FP16, tag=f"kt{h}", name=f"kt{h}")
          for h in range(2)]
    vp = io_pool.tile([P, NKT, D + 1], FP16, tag="vp", name="vp")
    msk = io_pool.tile([P, NKT], F32, tag="msk", name="msk")

    # ---- optional in-NEFF repetition of the ENTIRE body (timing) -----------
    loop_cm = None
    if reps > 1:
        loop_cm = tc.For_i(0, reps, 1, hint_engines=(
            mybir.EngineType.PE, mybir.EngineType.Activation,
            mybir.EngineType.DVE, mybir.EngineType.Pool))
        loop_cm.__enter__()

    # ---- prologue: load + transpose Q and K (scaled), build V' -------------
    stage_pool = ctx.enter_context(tc.tile_pool(name="stage", bufs=1))
    qn = stage_pool.tile([P, NKT, D], F32, tag="qn", name="qn")
    kn = stage_pool.tile([P, NKT, D], F32, tag="kn", name="kn")
    ones = stage_pool.tile([P, NKT], F32, tag="ones", name="ones")

    q_tiled = q_ap.rearrange("(t p) d -> p t d", p=P)
    k_tiled = k_ap.rearrange("(t p) d -> p t d", p=P)
    v_tiled = v_ap.rearrange("(t p) d -> p t d", p=P)
    mask_tiled = mask_ap.rearrange("1 (t p) -> p t", p=P)

    nc.sync.dma_start(qn[:], q_tiled)
    nc.sync.dma_start(kn[:], k_tiled)
    nc.sync.dma_start(msk, mask_tiled)
    nc.gpsimd.memset(ones, 1.0)
    nc.vector.tensor_copy(vp[:, :, D], ones)
    vn = stage_pool.tile([P, NKT, D], F32, tag="vn", name="vn")
    nc.sync.dma_start(vn[:], v_tiled)
    nc.vector.tensor_copy(vp[:, :, 0:D], vn)

    with tc.tile_pool(name="tpsum", bufs=4, space="PSUM") as tpsum_pool:
        # Transpose 4 input tiles [128, 64] into one psum bank [64, 512],
        # drain with the CSCALE prescale fused (alternate DVE/ACT).
        for half in range(2):
            for src_i, (src, dst) in enumerate(((qn, qt), (kn, kt))):
                for c in range(4):
                    ps = tpsum_pool.tile([D, 4 * P], F32, tag="tps", name="tps")
                    for j in range(4):
                        t = half * (NKT // 2) + c * 4 + j
                        nc.tensor.transpose(ps[:, j * P:(j + 1) * P],
                                            src[:, t, :], ident)
                    dcol = c * 4 * P
                    if (c + src_i) % 2 == 0:
                        nc.vector.tensor_scalar_mul(
                            dst[half][0:D, dcol:dcol + 4 * P], ps, CSCALE)
                    else:
                        nc.scalar.mul(
                            dst[half][0:D, dcol:dcol + 4 * P], ps, CSCALE)
            nc.sync.dma_start(qt[half][D:P, :], qt[half][0:D, :])
            nc.sync.dma_start(kt[half][D:P, :], kt[half][0:D, :])

    # ---- main loop ---------------------------------------------------------
    pv_bufs = PV_BUFS if PV_BUFS is not None else (1 if PV_MODE == "fast4" else 2)
    pt_pool = ctx.enter_context(tc.tile_pool(name="pt", bufs=3))
    sc_pool = ctx.enter_context(tc.tile_pool(name="sc", bufs=SC_BUFS,
                                             space="PSUM"))
    m4_pool = ctx.enter_context(tc.tile_pool(name="m4", bufs=2))
    pv_pool = ctx.enter_context(tc.tile_pool(
        name="pv", bufs=pv_bufs, space="PSUM"))
    osb_pool = ctx.enter_context(tc.tile_pool(name="osb", bufs=2))
    scal_pool = ctx.enter_context(tc.tile_pool(name="scal", bufs=4))

    # psum column offset of sub-chain accumulators & total pv tile columns
    if PV_MODE == "safe":
        PV_COLS = NSUB * (D + 1)
        pv_off = lambda sub: sub * (D + 1)
    elif PV_MODE == "fast2":
        PV_COLS = 2 * 512
        pv_off = lambda sub: (sub % 2) * 512 + (sub // 2) * (D + 1)
    else:  # fast4
        PV_COLS = 4 * 512
        pv_off = lambda sub: sub * 512

    out_tiled = out_ap.rearrange("(t p) d -> p t d", p=P)

    pt_tiles = {}   # qc -> ptt tile [P, NKT, QCHUNK] fp16
    pv_tiles = {}   # qc -> pv psum tile [P, NSUB, D+1]

    sink = None
    if SKIP_PV:
        sink = io_pool.tile([P, NQC, 32], FP16, tag="sink", name="sink")
    fixed_pt = None
    if SKIP_QKEXP:
        _fpt = io_pool.tile([P, NKT, QCHUNK], FP16, tag="fpt", name="fpt")
        for j in range(NKT // 2):
            nc.gpsimd.tensor_copy(
                _fpt.rearrange("p a b -> p (a b)")[:, j * 1024:(j + 1) * 1024],
                qn.rearrange("p a b -> p (a b)")[:, 0:1024])
        fixed_pt = {"A": _fpt, "D": _fpt}

    # chunk -> (engine-class tile key, slab offset within that tile);
    # kt -> same, for PV lhsT lookup. Engine classes: ACT ('A') vs DVE
    # ('V'/'W'), each writing its OWN pt tile so the two exp engines never
    # share a write target (tile-granular dep tracking would serialize them).
    chunk_starts = []
    kt_map = {}
    n_slab = {"A": 0, "D": 0}
    _kt = 0
    for _eng, _cl in EXP_PATTERN:
        key = "A" if _eng == "A" else "D"
        chunk_starts.append((key, n_slab[key]))
        for _j in range(_cl):
            kt_map[_kt + _j] = (key, n_slab[key] + _j)
        n_slab[key] += _cl
        _kt += _cl
    assert _kt == NKT

    def emit_exp_chunk(qc, c):
        q0 = qc * QCHUNK
        qt_half = qt[(2 * q0) // S]
        qcol = q0 % (S // 2)
        if c == 0:
            tiles = {}
            for key, cnt in n_slab.items():
                if cnt:
                    tiles[key] = pt_pool.tile([P, cnt, QCHUNK], FP16,
                                              tag=f"ptt{key}", name=f"ptt{key}")
            pt_tiles[qc] = tiles
        tiles = pt_tiles[qc]
        eng, clen = EXP_PATTERN[c]
        key, slab0 = chunk_starts[c]
        kt0 = sum(cl for _, cl in EXP_PATTERN[:c])
        scs = sc_pool.tile([P, clen * QCHUNK], F32, tag="sc", name="sc")
        for jj in range(clen):
            k_tile = kt0 + jj
            h = k_tile % 2
            kt_half = kt[(k_tile * P * 2) // S]
            kcol = (k_tile * P) % (S // 2)
            nc.tensor.matmul(
                scs[:, jj * QCHUNK:(jj + 1) * QCHUNK],
                lhsT=kt_half[h * D:(h + 1) * D, kcol:kcol + P],
                rhs=qt_half[h * D:(h + 1) * D, qcol:qcol + QCHUNK],
                start=True, stop=True,
            )
        dst = tiles[key][:, slab0:slab0 + clen, :]
        if eng == "A":
            nc.scalar.activation(
                dst.rearrange("p a b -> p (a b)"), scs,
                mybir.ActivationFunctionType.Exp, scale=ACT_SCALE)
        elif eng == "W":
            nc.vector._custom_dve(
                EXP16W, out=dst.rearrange("p a b -> p (a b)").bitcast(I16),
                in0=scs, in1=cstw,
                s0=CBIG, s1=KW_C1, imm2=KW_C2)
        else:
            m4 = m4_pool.tile([P, clen * QCHUNK], F32, tag="m4", name="m4")
            nc.vector._custom_dve(
                EXP16A, out=m4, in0=scs, in1=cst,
                s0=CBIG, s1=K_C1, imm2=K_C2)
            nc.vector._custom_dve(
                EXP16B,
                out=dst.bitcast(I16),
                in0=m4.rearrange("p (a b) -> p a b", a=clen),
                in1=scs.rearrange("p (a b) -> p a b", a=clen),
                s0=CBIG, s1=K_CA)

    def pv_mm(pv_ps, tiles, k_tile, sub):
        off = pv_off(sub)
        key, idx = kt_map[k_tile]
        nc.tensor.matmul(
            pv_ps[:, off:off + D + 1],
            lhsT=tiles[key][:, idx, sub * P:(sub + 1) * P],
            rhs=vp[:, k_tile, :],
            start=(k_tile == 0), stop=(k_tile == NKT - 1),
            skip_group_check=True,
        )

    def pv_pieces(qc):
        """List of ~16-MM emission closures in a chain-legal order."""
        tiles = fixed_pt if SKIP_QKEXP else pt_tiles.pop(qc)
        if SKIP_PV:
            def piece():
                for j, t in enumerate(tiles.values()):
                    nc.gpsimd.tensor_copy(sink[:, qc, 16 * j:16 * j + 16],
                                          t[:, 0, 0:16])
                if qc == NQC - 1:
                    nc.sync.dma_start(
                        out_tiled[:, 0:2, :],
                        sink.bitcast(F32).rearrange(
                            "p a b -> p (a b)").rearrange(
                            "p (t d) -> p t d", t=2))
            return [piece]
        pv_ps = pv_pool.tile([P, PV_COLS], F32, tag="pv", name="pv")
        pv_tiles[qc] = pv_ps
        pieces = []
        if PV_MODE == "safe":
            # contiguous chain per sub, split in kt halves
            for sub in range(NSUB):
                for half in range(2):
                    def piece(sub=sub, half=half):
                        for k_tile in range(half * 16, half * 16 + 16):
                            pv_mm(pv_ps, tiles, k_tile, sub)
                    pieces.append(piece)
        elif PV_MODE == "fast2":
            # two chains interleaved across two banks, two passes
            for pair in range(2):
                for quarter in range(4):
                    def piece(pair=pair, quarter=quarter):
                        for k_tile in range(quarter * 8, quarter * 8 + 8):
                            for sub in (2 * pair, 2 * pair + 1):
                                pv_mm(pv_ps, tiles, k_tile, sub)
                    pieces.append(piece)
        else:  # fast4: all four chains interleaved kt-outer
            for eighth in range(8):
                def piece(eighth=eighth):
                    for k_tile in range(eighth * 4, eighth * 4 + 4):
                        for sub in range(NSUB):
                            pv_mm(pv_ps, tiles, k_tile, sub)
                pieces.append(piece)
        return pieces

    def emit_epilogue(qc):
        pv_ps = pv_tiles.pop(qc)
        rs = scal_pool.tile([P, 2 * NSUB], F32, tag="rs", name="rs")
        for sub in range(NSUB):
            nc.vector.reciprocal(rs[:, sub:sub + 1],
                                 pv_ps[:, pv_off(sub) + D:pv_off(sub) + D + 1])
        nc.vector.tensor_mul(rs[:, NSUB:2 * NSUB], rs[:, 0:NSUB],
                             msk[:, qc * NSUB:(qc + 1) * NSUB])
        osb = osb_pool.tile([P, NSUB, D], F32, tag="osb", name="osb")
        for sub in range(NSUB):
            src = pv_ps[:, pv_off(sub):pv_off(sub) + D]
            if sub % 2 == 0:
                nc.scalar.activation(
                    osb[:, sub, :], src,
                    mybir.ActivationFunctionType.Copy,
                    scale=rs[:, NSUB + sub:NSUB + sub + 1],
                )
            else:
                nc.vector.tensor_scalar(
                    osb[:, sub, :], src,
                    rs[:, NSUB + sub:NSUB + sub + 1], None,
                    mybir.AluOpType.mult,
                )
        nc.sync.dma_start(
            out_tiled[:, qc * NSUB:(qc + 1) * NSUB, :], osb)

    n_chunks = len(EXP_PATTERN)
    if PIPELINE:
        # PV(s-1) pieces slot between QK/exp chunk emissions of step s so the
        # PE works through PV while ACT/DVE chew on exp(s).
        for s in range(NQC + 1):
            pieces = pv_pieces(s - 1) if s >= 1 else []
            if s < NQC and not SKIP_QKEXP:
                done = 0
                for c in range(n_chunks):
                    emit_exp_chunk(s, c)
                    # spread PV pieces evenly across the chunk emissions
                    want = ((c + 1) * len(pieces)) // n_chunks
                    while done < want:
                        pieces[done]()
                        done += 1
                while done < len(pieces):
                    pieces[done]()
                    done += 1
            else:
                for piece in pieces:
                    piece()
            if s >= 1 and not SKIP_PV:
                emit_epilogue(s - 1)
    else:
        for s in range(NQC):
            if not SKIP_QKEXP:
                for c in range(n_chunks):
                    emit_exp_chunk(s, c)
            for piece in pv_pieces(s):
                piece()
            if not SKIP_PV:
                emit_epilogue(s)

    if loop_cm is not None:
        loop_cm.__exit__(None, None, None)


def build_program(reps=1):
    nc = bacc.Bacc("TRN2", target_bir_lowering=False, debug=False,
                   num_devices=B)
    q = nc.declare_dram_parameter("q", [S, D], F32, isOutput=False).ap()
    k = nc.declare_dram_parameter("k", [S, D], F32, isOutput=False).ap()
    v = nc.declare_dram_parameter("v", [S, D], F32, isOutput=False).ap()
    mask = nc.declare_dram_parameter("mask", [1, S], F32, isOutput=False).ap()
    out = nc.declare_dram_parameter("out", [S, D], F32, isOutput=True).ap()

    with tile.TileContext(nc) as tc, ExitStack() as ctx:
        build_attention(ctx, tc, q, k, v, mask, out, reps=reps)
    nc.compile()
    return nc


_NC_CACHE = None


def _get_nc():
    global _NC_CACHE
    if _NC_CACHE is None:
        _NC_CACHE = build_program()
    return _NC_CACHE


def make_in_maps(q, k, v, mask):
    return [
        {
            "q": np.ascontiguousarray(q[b], dtype=np.float32),
            "k": np.ascontiguousarray(k[b], dtype=np.float32),
            "v": np.ascontiguousarray(v[b], dtype=np.float32),
            "mask": np.ascontiguousarray(mask[b][None, :], dtype=np.float32),
        }
        for b in range(B)
    ]


def kernel(q, k, v, mask):
    from concourse.bass_utils import run_bass_kernel_spmd

    nc = _get_nc()
    res = run_bass_kernel_spmd(nc, make_in_maps(q, k, v, mask), list(range(B)))
    return np.stack([res.results[b]["out"] for b in range(B)])


if __name__ == "__main__":
    rng = np.random.default_rng(0)
    q = rng.standard_normal((B, S, D), dtype=np.float32)
    k = rng.standard_normal((B, S, D), dtype=np.float32)
    v = rng.standard_normal((B, S, D), dtype=np.float32)
    mask = np.ones((B, S), dtype=np.float32)
    out = kernel(q, k, v, mask)
    print("out", out.shape, out.dtype, float(np.abs(out).max()))


# revision 8
# speedup vs baseline: 1.6567x; 1.3017x over previous
"""Trainium2 Bass kernel v2: batched attention (B=8, S=4096, D=64), fp32.

out[b] = softmax(q[b] @ k[b].T / sqrt(D), axis=keys) @ v[b] * mask[b, :, None]

Data-parallel over batch: one batch element per NeuronCore, 8 cores.

Per-core algorithm:
  1. Q, K transposed to d-major fp16, PRESCALED by sqrt(2^10*log2e/8) each so
     the QK^T psum value is s = 2^10 * log2(e) * (q.k)/8 -- the exact fixed-
     point argument both exp paths want.
  2. scoresT[k, q] slabs ([128 keys x 512 q] psum) via one-shot half-array
     matmul pairs (even/odd k-tiles on the two 64-row PE halves).
  3. exp split across TWO engines per 3-slab chunk, alternating:
     - ScalarE ('A'): PT = Exp(ln2/1024 * s) -> fp16
     - VectorE ('V'): custom 2-op fast-exp: I16 = 2^10*(15+round(t)+f+h(f)),
       h fitted over {1, f, f^2, |f|}, int16 store == fp16 bits of e^x
       (bit-exact vs numpy emulation on HW; max P rel err 2.9e-3)
     - VectorE ('W'): 1-op quad-only variant (max P rel err ~1.6%, cheap).
     A/W/V mix balances ACT vs DVE throughput against the 2e-2 error gate;
     ACT and DVE write separate PT tiles (per-tile dep tracking).
  4. PV swapped: PT subtile [128k x 128q] STATIONARY, moving V' [128k x 65]
     (V padded with a ones column -> denominator lands in psum col 64).
     Accumulate over 32 k-tiles into psum [128q, 4, 65] per 512-q chunk;
     each sub-chain contiguous (interleaving start/stop groups within one
     psum bank corrupts accumulation; across banks it is fine but slower
     here). PV pieces are emitted BETWEEN exp chunks of the next q-chunk so
     the PE works while ACT/DVE chew on exp.
  5. Epilogue (no PE): rec = 1/denom (DVE), rm = rec*mask (DVE),
     osb = psum * rm (GpSimd), DMA out. Output is already [q, d] natural.
"""

import sys

if "/opt/trn_rl_repo" not in sys.path:
    sys.path.insert(0, "/opt/trn_rl_repo")

import math
from contextlib import ExitStack

import numpy as np

import concourse.bass as bass
import concourse.mybir as mybir
import concourse.tile as tile
from concourse import bacc
from concourse.masks import make_identity

F32 = mybir.dt.float32
FP16 = mybir.dt.float16
I16 = mybir.dt.int16

B = 8          # batch == number of cores
S = 4096       # sequence length
D = 64         # head dim
P = 128        # partitions
NKT = S // P   # 32 k-tiles of 128 keys
QCHUNK = 512   # query chunk (psum-bank-sized scores slabs)
NQC = S // QCHUNK          # 8 query chunks
NSUB = QCHUNK // P         # 4 q-subtiles per chunk

# ---- fast-exp fixed-point scaling ------------------------------------------
# psum scores s = ALPHA * (q.k); t = s/2^10 = log2(e) * (q.k)/8
ALPHA = 1024.0 * math.log2(math.e) / 8.0
CSCALE = math.sqrt(ALPHA)            # folded into each of Q and K
ACT_SCALE = math.log(2.0) / 1024.0   # ScalarE: exp(ACT_SCALE * s) = e^{qk/8}
CBIG = float(3 * 2**32)              # fp32 round-to-nearest at 2^10 granularity
# minimax fit of h(f) = {f>=0: 2^f-1-f ; f<0: 2^(f+1)-2-f} on [-1/2,1/2]
# over {1, f, f^2, |f|}: max resid 2.75e-3
C_H0 = -2.37450629e-04
C_H1 = 5.50407949e-03
C_H2 = 3.44100112e-01
C_HA = -3.43148039e-01
K_C1 = float(C_H2 / 1024.0)            # op1 s1
K_C2 = float(1.0 + C_H1)               # op1 imm2
K_C0K = float((C_H0 + 15.0) * 1024.0)  # op1 C3->Latch(Src1) [P,1] value
K_CA = float(C_HA)                     # op2 s1 literal

# per-q-chunk exp engine pattern: (engine, nslabs) summing NKT=32.
# 'A' = ScalarE activation, 'V' = VectorE custom 2-op fast-exp
EXP_PATTERN = ([("A", 2), ("W", 2)] * 6 + [("A", 2), ("V", 2)] * 2)
PIPELINE = True
SKIP_PV = False    # diagnostic: drop PV+epilogue (QK+exp pipeline only)
SKIP_QKEXP = False  # diagnostic: PV reads prologue-seeded PT; no QK/exp
# PV chain layout: "safe"  = sub-outer, contiguous chains, 1-bank psum tile
#                  "fast2" = 2 banks, pairs of chains interleaved kt-outer
#                  "fast4" = 4 banks, all 4 chains interleaved kt-outer
PV_MODE = "safe"
SC_BUFS = 3
PV_BUFS = 2  # default: 1 for fast4 else 2

# ---- custom DVE op registration --------------------------------------------
from concourse.dve_spec import (
    Spec, Src0, Src1, C0, C1, C2, C3, Zero, maxx,
    lower as _dve_lower, _has_src1, _spill_c3_to_src1,
)
from concourse.dve_ops import (
    DveOp, OPS as _DVE_OPS, CUSTOM_DVE_SPECS as _DVE_SPECS,
    _SUB_OPCODE_FOR_NAME as _DVE_ROWS,
)
from concourse.dve_uop import DveOpSpec


def _register_op(name, body):
    if name in _DVE_ROWS:
        return next(op for op in _DVE_OPS if op.name == name)
    spec = Spec(body=_spill_c3_to_src1(body))
    uops = _dve_lower(spec, ver="v3")
    row = 1 + len(_DVE_OPS)
    assert row < 0x20
    sha = DveOpSpec(name=name, opcode=row, uops=uops,
                    rd1_en=_has_src1(spec)).sha("v3")
    op = DveOp(name=name, spec=spec, subdim=False, uops_sha={"v3": sha})
    _DVE_OPS.append(op)
    _DVE_SPECS[name] = spec
    _DVE_ROWS[name] = row
    return op


def _exp16a_body():
    # m4 = ((F*C1 + C2)*F) + C3 ; F = s - ((s + C0) - C0)
    R = Src0 + C0
    G = R - C0
    F = Src0 - G
    return ((F * C1) + C2) * F + C3


def _exp16b_body():
    # I = (|F| * C1) + Src0 + G   (Src1 = scores stream, rank-3)
    R = Src1 + C0
    G = R - C0
    F = Src1 - G
    A = maxx(F, Zero - F)
    return (A * C1) + Src0 + G


def _exp16w_body():
    # single-op quadratic-only variant (no |f| kink term): cheap but coarser
    # (max P rel err ~1.6%); I = ((F*C1 + C2)*F + C3) + G
    R = Src0 + C0
    G = R - C0
    F = Src0 - G
    return (((F * C1) + C2) * F + C3) + G


EXP16A = _register_op("ANT_EXP16A", _exp16a_body())
EXP16B = _register_op("ANT_EXP16B", _exp16b_body())
EXP16W = _register_op("ANT_EXP16W", _exp16w_body())

# quad-only minimax coefficients (basis {1, f, f^2}), resid 2.25e-2
W_H0 = -0.02251146
W_H1 = 0.01399606
W_H2 = -0.31515363
KW_C1 = float(W_H2 / 1024.0)
KW_C2 = float(1.0 + W_H1)
KW_C0K = float((W_H0 + 15.0) * 1024.0)


# ---- kernel build ----------------------------------------------------------
def build_attention(ctx: ExitStack, tc: tile.TileContext,
                    q_ap, k_ap, v_ap, mask_ap, out_ap, reps=1):
    nc = tc.nc

    const_pool = ctx.enter_context(tc.tile_pool(name="const", bufs=1))
    io_pool = ctx.enter_context(tc.tile_pool(name="io", bufs=1))

    ident = const_pool.tile([P, P], F32, tag="ident", name="ident")
    make_identity(nc, ident)
    cst = const_pool.tile([P, 1], F32, tag="cst", name="cst")
    nc.gpsimd.memset(cst, K_C0K)
    cstw = const_pool.tile([P, 1], F32, tag="cstw", name="cstw")
    nc.gpsimd.memset(cstw, KW_C0K)

    # persistent SBUF tensors
    qt = [io_pool.tile([P, S // 2], FP16, tag=f"qt{h}", name=f"qt{h}")
          for h in range(2)]
    kt = [io_pool.tile([P, S // 2], FP16, tag=f"kt{h}", name=f"kt{h}")
          for h in range(2)]
    vp = io_pool.tile([P, NKT, D + 1], FP16, tag="vp", name="vp")
    msk = io_pool.tile([P, NKT], F32, tag="msk", name="msk")

    # ---- optional in-NEFF repetition of the ENTIRE body (timing) -----------
    loop_cm = None
    if reps > 1:
        loop_cm = tc.For_i(0, reps, 1, hint_engines=(
            mybir.EngineType.PE, mybir.EngineType.Activation,
            mybir.EngineType.DVE, mybir.EngineType.Pool))
        loop_cm.__enter__()

    # ---- prologue: load + transpose Q and K (scaled), build V' -------------
    stage_pool = ctx.enter_context(tc.tile_pool(name="stage", bufs=1))
    qn = stage_pool.tile([P, NKT, D], F32, tag="qn", name="qn")
    kn = stage_pool.tile([P, NKT, D], F32, tag="kn", name="kn")
    ones = stage_pool.tile([P, NKT], F32, tag="ones", name="ones")

    q_tiled = q_ap.rearrange("(t p) d -> p t d", p=P)
    k_tiled = k_ap.rearrange("(t p) d -> p t d", p=P)
    v_tiled = v_ap.rearrange("(t p) d -> p t d", p=P)
    mask_tiled = mask_ap.rearrange("1 (t p) -> p t", p=P)

    nc.sync.dma_start(qn[:], q_tiled)
    nc.sync.dma_start(kn[:], k_tiled)
    nc.sync.dma_start(msk, mask_tiled)
    nc.gpsimd.memset(ones, 1.0)
    nc.vector.tensor_copy(vp[:, :, D], ones)
    vn = stage_pool.tile([P, NKT, D], F32, tag="vn", name="vn")
    nc.sync.dma_start(vn[:], v_tiled)
    nc.vector.tensor_copy(vp[:, :, 0:D], vn)

    with tc.tile_pool(name="tpsum", bufs=4, space="PSUM") as tpsum_pool:
        # Transpose 4 input tiles [128, 64] into one psum bank [64, 512],
        # drain with the CSCALE prescale fused (alternate DVE/ACT).
        for half in range(2):
            for src_i, (src, dst) in enumerate(((qn, qt), (kn, kt))):
                for c in range(4):
                    ps = tpsum_pool.tile([D, 4 * P], F32, tag="tps", name="tps")
                    for j in range(4):
                        t = half * (NKT // 2) + c * 4 + j
                        nc.tensor.transpose(ps[:, j * P:(j + 1) * P],
                                            src[:, t, :], ident)
                    dcol = c * 4 * P
                    if (c + src_i) % 2 == 0:
                        nc.vector.tensor_scalar_mul(
                            dst[half][0:D, dcol:dcol + 4 * P], ps, CSCALE)
                    else:
                        nc.scalar.mul(
                            dst[half][0:D, dcol:dcol + 4 * P], ps, CSCALE)
            nc.sync.dma_start(qt[half][D:P, :], qt[half][0:D, :])
            nc.sync.dma_start(kt[half][D:P, :], kt[half][0:D, :])

    # ---- main loop ---------------------------------------------------------
    pv_bufs = PV_BUFS if PV_BUFS is not None else (1 if PV_MODE == "fast4" else 2)
    pt_pool = ctx.enter_context(tc.tile_pool(name="pt", bufs=3))
    sc_pool = ctx.enter_context(tc.tile_pool(name="sc", bufs=SC_BUFS,
                                             space="PSUM"))
    m4_pool = ctx.enter_context(tc.tile_pool(name="m4", bufs=2))
    pv_pool = ctx.enter_context(tc.tile_pool(
        name="pv", bufs=pv_bufs, space="PSUM"))
    osb_pool = ctx.enter_context(tc.tile_pool(name="osb", bufs=2))
    scal_pool = ctx.enter_context(tc.tile_pool(name="scal", bufs=4))

    # psum column offset of sub-chain accumulators & total pv tile columns
    if PV_MODE == "safe":
        PV_COLS = NSUB * (D + 1)
        pv_off = lambda sub: sub * (D + 1)
    elif PV_MODE == "fast2":
        PV_COLS = 2 * 512
        pv_off = lambda sub: (sub % 2) * 512 + (sub // 2) * (D + 1)
    else:  # fast4
        PV_COLS = 4 * 512
        pv_off = lambda sub: sub * 512

    out_tiled = out_ap.rearrange("(t p) d -> p t d", p=P)

    pt_tiles = {}   # qc -> ptt tile [P, NKT, QCHUNK] fp16
    pv_tiles = {}   # qc -> pv psum tile [P, NSUB, D+1]

    sink = None
    if SKIP_PV:
        sink = io_pool.tile([P, NQC, 32], FP16, tag="sink", name="sink")
    fixed_pt = None
    if SKIP_QKEXP:
        _fpt = io_pool.tile([P, NKT, QCHUNK], FP16, tag="fpt", name="fpt")
        for j in range(NKT // 2):
            nc.gpsimd.tensor_copy(
                _fpt.rearrange("p a b -> p (a b)")[:, j * 1024:(j + 1) * 1024],
                qn.rearrange("p a b -> p (a b)")[:, 0:1024])
        fixed_pt = {"A": _fpt, "D": _fpt}

    # chunk -> (engine-class tile key, slab offset within that tile);
    # kt -> same, for PV lhsT lookup. Engine classes: ACT ('A') vs DVE
    # ('V'/'W'), each writing its OWN pt tile so the two exp engines never
    # share a write target (tile-granular dep tracking would serialize them).
    chunk_starts = []
    kt_map = {}
    n_slab = {"A": 0, "D": 0}
    _kt = 0
    for _eng, _cl in EXP_PATTERN:
        key = "A" if _eng == "A" else "D"
        chunk_starts.append((key, n_slab[key]))
        for _j in range(_cl):
            kt_map[_kt + _j] = (key, n_slab[key] + _j)
        n_slab[key] += _cl
        _kt += _cl
    assert _kt == NKT

    def emit_exp_chunk(qc, c):
        q0 = qc * QCHUNK
        qt_half = qt[(2 * q0) // S]
        qcol = q0 % (S // 2)
        if c == 0:
            tiles = {}
            for key, cnt in n_slab.items():
                if cnt:
                    tiles[key] = pt_pool.tile([P, cnt, QCHUNK], FP16,
                                              tag=f"ptt{key}", name=f"ptt{key}")
            pt_tiles[qc] = tiles
        tiles = pt_tiles[qc]
        eng, clen = EXP_PATTERN[c]
        key, slab0 = chunk_starts[c]
        kt0 = sum(cl for _, cl in EXP_PATTERN[:c])
        scs = sc_pool.tile([P, clen * QCHUNK], F32, tag="sc", name="sc")
        for jj in range(clen):
            k_tile = kt0 + jj
            h = k_tile % 2
            kt_half = kt[(k_tile * P * 2) // S]
            kcol = (k_tile * P) % (S // 2)
            nc.tensor.matmul(
                scs[:, jj * QCHUNK:(jj + 1) * QCHUNK],
                lhsT=kt_half[h * D:(h + 1) * D, kcol:kcol + P],
                rhs=qt_half[h * D:(h + 1) * D, qcol:qcol + QCHUNK],
                start=True, stop=True,
            )
        dst = tiles[key][:, slab0:slab0 + clen, :]
        if eng == "A":
            nc.scalar.activation(
                dst.rearrange("p a b -> p (a b)"), scs,
                mybir.ActivationFunctionType.Exp, scale=ACT_SCALE)
        elif eng == "W":
            nc.vector._custom_dve(
                EXP16W, out=dst.rearrange("p a b -> p (a b)").bitcast(I16),
                in0=scs, in1=cstw,
                s0=CBIG, s1=KW_C1, imm2=KW_C2)
        else:
            m4 = m4_pool.tile([P, clen * QCHUNK], F32, tag="m4", name="m4")
            nc.vector._custom_dve(
                EXP16A, out=m4, in0=scs, in1=cst,
                s0=CBIG, s1=K_C1, imm2=K_C2)
            nc.vector._custom_dve(
                EXP16B,
                out=dst.bitcast(I16),
                in0=m4.rearrange("p (a b) -> p a b", a=clen),
                in1=scs.rearrange("p (a b) -> p a b", a=clen),
                s0=CBIG, s1=K_CA)

    def pv_mm(pv_ps, tiles, k_tile, sub):
        off = pv_off(sub)
        key, idx = kt_map[k_tile]
        nc.tensor.matmul(
            pv_ps[:, off:off + D + 1],
            lhsT=tiles[key][:, idx, sub * P:(sub + 1) * P],
            rhs=vp[:, k_tile, :],
            start=(k_tile == 0), stop=(k_tile == NKT - 1),
            skip_group_check=True,
        )

    def pv_pieces(qc):
        """List of ~16-MM emission closures in a chain-legal order."""
        tiles = fixed_pt if SKIP_QKEXP else pt_tiles.pop(qc)
        if SKIP_PV:
            def piece():
                for j, t in enumerate(tiles.values()):
                    nc.gpsimd.tensor_copy(sink[:, qc, 16 * j:16 * j + 16],
                                          t[:, 0, 0:16])
                if qc == NQC - 1:
                    nc.sync.dma_start(
                        out_tiled[:, 0:2, :],
                        sink.bitcast(F32).rearrange(
                            "p a b -> p (a b)").rearrange(
                            "p (t d) -> p t d", t=2))
            return [piece]
        pv_ps = pv_pool.tile([P, PV_COLS], F32, tag="pv", name="pv")
        pv_tiles[qc] = pv_ps
        pieces = []
        if PV_MODE == "safe":
            # contiguous chain per sub, split in kt quarters: 16 pieces of
            # 8 MMs -> one piece per exp chunk for a smooth PE interleave
            for sub in range(NSUB):
                for quarter in range(4):
                    def piece(sub=sub, quarter=quarter):
                        for k_tile in range(quarter * 8, quarter * 8 + 8):
                            pv_mm(pv_ps, tiles, k_tile, sub)
                    pieces.append(piece)
        elif PV_MODE == "fast2":
            # two chains interleaved across two banks, two passes
            for pair in range(2):
                for quarter in range(4):
                    def piece(pair=pair, quarter=quarter):
                        for k_tile in range(quarter * 8, quarter * 8 + 8):
                            for sub in (2 * pair, 2 * pair + 1):
                                pv_mm(pv_ps, tiles, k_tile, sub)
                    pieces.append(piece)
        else:  # fast4: all four chains interleaved kt-outer
            for eighth in range(8):
                def piece(eighth=eighth):
                    for k_tile in range(eighth * 4, eighth * 4 + 4):
                        for sub in range(NSUB):
                            pv_mm(pv_ps, tiles, k_tile, sub)
                pieces.append(piece)
        return pieces

    def emit_epilogue(qc):
        pv_ps = pv_tiles.pop(qc)
        rs = scal_pool.tile([P, 2 * NSUB], F32, tag="rs", name="rs")
        for sub in range(NSUB):
            nc.vector.reciprocal(rs[:, sub:sub + 1],
                                 pv_ps[:, pv_off(sub) + D:pv_off(sub) + D + 1])
        nc.vector.tensor_mul(rs[:, NSUB:2 * NSUB], rs[:, 0:NSUB],
                             msk[:, qc * NSUB:(qc + 1) * NSUB])
        osb = osb_pool.tile([P, NSUB, D], F32, tag="osb", name="osb")
        for sub in range(NSUB):
            src = pv_ps[:, pv_off(sub):pv_off(sub) + D]
            if sub % 2 == 0:
                nc.scalar.activation(
                    osb[:, sub, :], src,
                    mybir.ActivationFunctionType.Copy,
                    scale=rs[:, NSUB + sub:NSUB + sub + 1],
                )
            else:
                nc.vector.tensor_scalar(
                    osb[:, sub, :], src,
                    rs[:, NSUB + sub:NSUB + sub + 1], None,
                    mybir.AluOpType.mult,
                )
        nc.sync.dma_start(
            out_tiled[:, qc * NSUB:(qc + 1) * NSUB, :], osb)

    n_chunks = len(EXP_PATTERN)
    if PIPELINE:
        # PV(s-1) pieces slot between QK/exp chunk emissions of step s so the
        # PE works through PV while ACT/DVE chew on exp(s).
        for s in range(NQC + 1):
            pieces = pv_pieces(s - 1) if s >= 1 else []
            if s < NQC and not SKIP_QKEXP:
                done = 0
                for c in range(n_chunks):
                    emit_exp_chunk(s, c)
                    # spread PV pieces evenly across the chunk emissions
                    want = ((c + 1) * len(pieces)) // n_chunks
                    while done < want:
                        pieces[done]()
                        done += 1
                while done < len(pieces):
                    pieces[done]()
                    done += 1
            else:
                for piece in pieces:
                    piece()
            if s >= 1 and not SKIP_PV:
                emit_epilogue(s - 1)
    else:
        for s in range(NQC):
            if not SKIP_QKEXP:
                for c in range(n_chunks):
                    emit_exp_chunk(s, c)
            for piece in pv_pieces(s):
                piece()
            if not SKIP_PV:
                emit_epilogue(s)

    if loop_cm is not None:
        loop_cm.__exit__(None, None, None)


def build_program(reps=1):
    nc = bacc.Bacc("TRN2", target_bir_lowering=False, debug=False,
                   num_devices=B)
    q = nc.declare_dram_parameter("q", [S, D], F32, isOutput=False).ap()
    k = nc.declare_dram_parameter("k", [S, D], F32, isOutput=False).ap()
    v = nc.declare_dram_parameter("v", [S, D], F32, isOutput=False).ap()
    mask = nc.declare_dram_parameter("mask", [1, S], F32, isOutput=False).ap()
    out = nc.declare_dram_parameter("out", [S, D], F32, isOutput=True).ap()

    with tile.TileContext(nc) as tc, ExitStack() as ctx:
        build_attention(ctx, tc, q, k, v, mask, out, reps=reps)
    nc.compile()
    return nc


_NC_CACHE = None


def _get_nc():
    global _NC_CACHE
    if _NC_CACHE is None:
        _NC_CACHE = build_program()
    return _NC_CACHE


def make_in_maps(q, k, v, mask):
    return [
        {
            "q": np.ascontiguousarray(q[b], dtype=np.float32),
            "k": np.ascontiguousarray(k[b], dtype=np.float32),
            "v": np.ascontiguousarray(v[b], dtype=np.float32),
            "mask": np.ascontiguousarray(mask[b][None, :], dtype=np.float32),
        }
        for b in range(B)
    ]


def kernel(q, k, v, mask):
    from concourse.bass_utils import run_bass_kernel_spmd

    nc = _get_nc()
    res = run_bass_kernel_spmd(nc, make_in_maps(q, k, v, mask), list(range(B)))
    return np.stack([res.results[b]["out"] for b in range(B)])


if __name__ == "__main__":
    rng = np.random.default_rng(0)
    q = rng.standard_normal((B, S, D), dtype=np.float32)
    k = rng.standard_normal((B, S, D), dtype=np.float32)
    v = rng.standard_normal((B, S, D), dtype=np.float32)
    mask = np.ones((B, S), dtype=np.float32)
    out = kernel(q, k, v, mask)
    print("out", out.shape, out.dtype, float(np.abs(out).max()))
